# revision 21
# baseline (speedup 1.0000x reference)
"""Multi-head attention (B=4, S=2048, D=1024, H=16) on 8 TRN2 NeuronCores.

Sharding: row-parallel over (batch, seq-half): core c owns batch b=c//2 and
query rows [half*1024, half*1024+1024), half=c%2. K/V are computed for the
full batch on each core (duplicated compute, zero collectives). Each core's
output rows are final => no all-reduce; host just stacks the 8 outputs.

Device pipeline per core (uniform SPMD; per-core differences live in in_maps):
  - QKV projections row-major (lhsT = xT tiles, rhs = W tiles); bias folded in
    as a K=1 ones-row matmul in the same PSUM accumulation group.
  - RoPE in row-major layout. Wq/Wk columns are pre-permuted per head to
    [evens | odds] so interleaved RoPE becomes rotate-half on free-dim slices.
  - PE transposes (regular matmul against identity) -> feature-major qT/kT.
  - scoresT[kpos, q] = (kT_h slice as lhsT) x (qT_h as rhs), K=64; exp on
    ScalarE straight from PSUM with fused 1/sqrt(64) scale; bf16 out.
  - AV: outT[d, q] accumulated over kpos tiles with v row-major as stationary;
    a ones column appended to v yields the softmax denominator for free.
  - normalize via VectorE reciprocal + GPSIMD partition-broadcast.
  - y = (aoT slices as lhsT) x W0 + b0 (same ones trick), f32 out, DMA.
"""

import numpy as np
import ml_dtypes

B, S, D, H = 4, 2048, 1024, 16
HD = D // H          # 64
P = 128
RQ = 1024            # query rows per core
RK = 2048            # key rows per core (full batch)
NQT = RQ // P        # 8
NKT = RK // P        # 16
NF = D // P          # 8
NPBF16 = np.dtype(ml_dtypes.bfloat16)

_CACHE = {}


def _host_prep():
    if "prep" in _CACHE:
        return _CACHE["prep"]
    perm = np.empty(D, np.int64)
    for h in range(H):
        base = h * HD
        perm[base:base + 32] = base + np.arange(0, HD, 2)
        perm[base + 32:base + 64] = base + np.arange(1, HD, 2)
    inv_freq = 1.0 / (10000.0 ** (np.arange(0, D, 2, dtype=np.float32) / D))
    pos = np.arange(S, dtype=np.float32)
    ang = pos[:, None] * inv_freq[None, :]          # [S, 512]
    cosT = np.cos(ang).astype(NPBF16)
    sinT = np.sin(ang).astype(NPBF16)
    ident = np.eye(P, dtype=NPBF16)
    _CACHE["prep"] = (perm, cosT, sinT, ident)
    return _CACHE["prep"]


def _build_module(phases="ABC"):
    key = f"nc_{phases}"
    if key in _CACHE:
        return _CACHE[key]
    import concourse.bacc as bacc
    import concourse.mybir as mybir
    import concourse.tile as tile
    from contextlib import ExitStack

    BF = mybir.dt.bfloat16
    F32 = mybir.dt.float32
    Exp = mybir.ActivationFunctionType.Exp
    MUL = mybir.AluOpType.mult
    SUB = mybir.AluOpType.subtract
    ADD = mybir.AluOpType.add

    nc = bacc.Bacc("TRN2", target_bir_lowering=False, debug=False,
                   enable_asserts=False, num_devices=8)

    xT_d = nc.dram_tensor("xT", [D, RK], BF, kind="ExternalInput").ap()
    cos_d = nc.dram_tensor("cosT", [RK, D // 2], BF, kind="ExternalInput").ap()
    sin_d = nc.dram_tensor("sinT", [RK, D // 2], BF, kind="ExternalInput").ap()
    wq_d = nc.dram_tensor("wq", [D, D], BF, kind="ExternalInput").ap()
    wk_d = nc.dram_tensor("wk", [D, D], BF, kind="ExternalInput").ap()
    wv_d = nc.dram_tensor("wv", [D, D], BF, kind="ExternalInput").ap()
    w0_d = nc.dram_tensor("w0", [D, D], BF, kind="ExternalInput").ap()
    bq_d = nc.dram_tensor("bq", [1, D], BF, kind="ExternalInput").ap()
    bk_d = nc.dram_tensor("bk", [1, D], BF, kind="ExternalInput").ap()
    bv_d = nc.dram_tensor("bv", [1, D], BF, kind="ExternalInput").ap()
    b0_d = nc.dram_tensor("b0", [1, D], BF, kind="ExternalInput").ap()
    id_d = nc.dram_tensor("ident", [P, P], BF, kind="ExternalInput").ap()
    y_d = nc.dram_tensor("y", [RQ, D], F32, kind="ExternalOutput").ap()

    def by_head(ap, inner):
        # [P, H*inner] -> [P, H, inner]
        return ap.rearrange("p (h e) -> p h e", h=H)

    with tile.TileContext(nc) as tc, ExitStack() as ctx:
        cpool = ctx.enter_context(tc.tile_pool(name="const", bufs=1))
        ident = cpool.tile([P, P], BF)
        nc.sync.dma_start(ident[:], id_d)
        ones = cpool.tile([1, P], BF)
        nc.vector.memset(ones[:], 1.0)
        brow = {}
        for nm, d in (("bq", bq_d), ("bk", bk_d), ("bv", bv_d), ("b0", b0_d)):
            t = cpool.tile([1, D], BF, tag=f"brow_{nm}", name=f"brow_{nm}")
            nc.sync.dma_start(t[:], d)
            brow[nm] = t
        kT = cpool.tile([P, NF * RK], BF)       # [128, 8*2048]
        # qz: per-head zero-padded qT. Head h data lives at partitions
        # [64*(h%2), 64*(h%2)+64), cols [h*RQ, (h+1)*RQ); the other 64
        # partitions stay zero so scores matmuls contract over K=128
        # (full PE array => no HAM utilization throttle).
        qz = cpool.tile([P, H * RQ], BF)
        for j in range(2):
            zrows = qz[64 * (1 - j):64 * (1 - j) + 64, :].rearrange(
                "p (h r) -> p h r", h=H)[:, j::2, :]
            nc.gpsimd.memset(zrows, 0.0)

        vpool = ctx.enter_context(tc.tile_pool(name="v", bufs=NKT))
        v_tiles = []

        # ---------------- Phase A: projections + rope + transpose ----------
        with tc.tile_pool(name="x", bufs=NF) as xpool, \
             tc.tile_pool(name="w", bufs=2 * NF) as wpool, \
             tc.tile_pool(name="cs", bufs=2) as cspool, \
             tc.tile_pool(name="ro", bufs=2) as ropool, \
             tc.tile_pool(name="pjp", bufs=2, space="PSUM") as pjp, \
             tc.tile_pool(name="tpp", bufs=2, space="PSUM") as tpp:

            xt = []
            for i in range(NF):
                t = xpool.tile([P, RK], BF, tag="xT", name=f"xT{i}")
                nc.sync.dma_start(t[:], xT_d[i * P:(i + 1) * P, :])
                xt.append(t)
            def _load_w(nm, d):
                tiles = []
                for i in range(NF):
                    t = wpool.tile([P, D], BF, tag="w", name=f"{nm}_{i}")
                    nc.sync.dma_start(t[:], d[i * P:(i + 1) * P, :])
                    tiles.append(t)
                return tiles

            def _proj(rt, wtiles, btile):
                """Row-tile rt -> PSUM [128 rows, 1024 feats], bias included."""
                pt = pjp.tile([P, D], F32, tag="pj", name="pj")
                for nb in range(2):
                    out = pt[:, nb * 512:(nb + 1) * 512]
                    for kin in range(NF):
                        nc.tensor.matmul(
                            out,
                            lhsT=xt[kin][:, rt * P:(rt + 1) * P],
                            rhs=wtiles[kin][:, nb * 512:(nb + 1) * 512],
                            start=(kin == 0), stop=False)
                    nc.tensor.matmul(
                        out, lhsT=ones[:],
                        rhs=btile[:, nb * 512:(nb + 1) * 512],
                        start=False, stop=True)
                return pt

            def _rope_transpose(rt, pt, dst):
                """RoPE psum row-tile -> bf16; PE-transpose into dst cols rt."""
                ct = cspool.tile([P, D // 2], BF, tag="cos", name="cos")
                st = cspool.tile([P, D // 2], BF, tag="sin", name="sin")
                nc.sync.dma_start(ct[:], cos_d[rt * P:(rt + 1) * P, :])
                nc.sync.dma_start(st[:], sin_d[rt * P:(rt + 1) * P, :])
                rp = ropool.tile([P, D], BF, tag="roped", name="roped")
                t1 = ropool.tile([P, D // 2], F32, tag="t1", name="t1")
                t2 = ropool.tile([P, D // 2], F32, tag="t2", name="t2")

                def EO(ap, j):  # j=0 evens-half, j=1 odds-half per head
                    return ap.rearrange("p (h t e) -> p h t e", h=H, t=2)[:, :, j:j + 1, :]

                ch = by_head(ct[:], 32)
                sh = by_head(st[:], 32)
                t1h = by_head(t1[:], 32)
                t2h = by_head(t2[:], 32)
                nc.vector.tensor_tensor(t1h, EO(pt[:], 0), ch, MUL)
                nc.vector.tensor_tensor(t2h, EO(pt[:], 1), sh, MUL)
                nc.vector.tensor_tensor(EO(rp[:], 0), t1h, t2h, SUB)
                nc.vector.tensor_tensor(t1h, EO(pt[:], 1), ch, MUL)
                nc.vector.tensor_tensor(t2h, EO(pt[:], 0), sh, MUL)
                nc.vector.tensor_tensor(EO(rp[:], 1), t1h, t2h, ADD)
                if dst is None:
                    # q: PE-transpose 8 slices, scatter head halves into qz
                    for g in range(2):
                        tp = tpp.tile([P, 4 * P], F32, tag="tp", name="tp")
                        for j in range(4):
                            f = g * 4 + j
                            nc.tensor.matmul(
                                tp[:, j * P:(j + 1) * P],
                                lhsT=rp[:, f * P:(f + 1) * P], rhs=ident[:],
                                start=True, stop=True)
                        # head 8g+2i+j sits at cols (8g+2i+j)*RQ, partitions 64j
                        for j in range(2):
                            out = qz[64 * j:64 * j + 64, :].rearrange(
                                "p (h r) -> p h r", h=H)[
                                :, 8 * g + j:min(8 * g + j + 8, H):2,
                                rt * P:(rt + 1) * P]
                            nc.vector.tensor_copy(
                                out, tp[64 * j:64 * j + 64, :].rearrange(
                                    "p (i r) -> p i r", i=4))
                else:
                    # k: DMA-transpose each 128x128 slice straight into kT,
                    # triggered from the (idle) ACT queue to keep the xbar
                    # transpose mode off the main SP input-load queue.
                    for f in range(NF):
                        nc.scalar.dma_start_transpose(
                            dst[:, f * RK + rt * P: f * RK + (rt + 1) * P],
                            rp[:, f * P:(f + 1) * P])

            def _vtile(rt, wv_t):
                pt = _proj(rt, wv_t, brow["bv"])
                vt = vpool.tile([P, H * (HD + 1)], BF, tag="v", name=f"v{rt}")
                vh = vt[:].rearrange("p (h e) -> p h e", h=H)
                nc.vector.tensor_copy(vh[:, :, 0:HD], by_head(pt[:], HD))
                nc.vector.memset(vh[:, :, HD:HD + 1], 1.0)
                v_tiles.append(vt)

            wq_t = _load_w("wq", wq_d)
            for rt in range(NQT):
                _rope_transpose(rt, _proj(rt, wq_t, brow["bq"]), None)
            wk_t = _load_w("wk", wk_d)
            wv_t = _load_w("wv", wv_d)
            for rt in range(NKT):
                _rope_transpose(rt, _proj(rt, wk_t, brow["bk"]), kT)
                _vtile(rt, wv_t)

        # ---------------- Phase B: attention, head pairs -------------------
        w0pool = ctx.enter_context(tc.tile_pool(name="w0", bufs=NF))
        aoT = w0pool.tile([P, NF * RQ], BF, tag="aoT", name="aoT", bufs=1)
        w0t = []
        for i in range(NF):
            t = w0pool.tile([P, D], BF, tag="w0", name=f"w0_{i}")
            nc.sync.dma_start(t[:], w0_d[i * P:(i + 1) * P, :])
            w0t.append(t)

        with tc.tile_pool(name="at", bufs=8) as apool, \
             tc.tile_pool(name="rec", bufs=2) as rcpool, \
             tc.tile_pool(name="stg", bufs=4) as stgpool, \
             tc.tile_pool(name="scp", bufs=2, space="PSUM") as scp, \
             tc.tile_pool(name="avp", bufs=2, space="PSUM") as avp:

            def _scores(ft, kt):
                """Scores + exp for both heads of pair ft at key tile kt.
                K=128 full-array matmuls: the complementary head's rows in
                qz are zero, so they contribute nothing."""
                ats = []
                for j in range(2):
                    h = 2 * ft + j
                    sp = scp.tile([P, RQ], F32, tag="sc", name="sc")
                    for nb in range(2):
                        nc.tensor.matmul(
                            sp[:, nb * 512:(nb + 1) * 512],
                            lhsT=kT[:, ft * RK + kt * P: ft * RK + (kt + 1) * P],
                            rhs=qz[:, h * RQ + nb * 512: h * RQ + (nb + 1) * 512],
                            start=True, stop=True)
                    at = apool.tile([P, RQ], BF, tag="at", name="at")
                    nc.scalar.activation(at[:], sp[:], Exp, scale=0.125)
                    ats.append(at)
                return ats

            def _av(hp, kt, avs, ats):
                for j in range(2):
                    h = 2 * hp + j
                    for nb in range(2):
                        nc.tensor.matmul(
                            avs[j][0:HD + 1, nb * 512:(nb + 1) * 512],
                            lhsT=v_tiles[kt][:, h * (HD + 1):(h + 1) * (HD + 1)],
                            rhs=ats[j][:, nb * 512:(nb + 1) * 512],
                            start=(kt == 0), stop=(kt == NKT - 1))

            for hp in range(H // 2 if "B" in phases else 0):
                ft = hp            # feature tile holding heads 2hp, 2hp+1
                avs = [avp.tile([P, RQ], F32, tag="av", name=f"av{_j}") for _j in range(2)]
                # software pipeline: scores/exp run one kt ahead of AV
                prev = _scores(ft, 0)
                for kt in range(1, NKT):
                    cur = _scores(ft, kt)
                    _av(hp, kt - 1, avs, prev)
                    prev = cur
                _av(hp, NKT - 1, avs, prev)
                for j in range(2):
                    # cheap copies release the AV psum tile; the rest of the
                    # normalize chain runs off the PE critical path. The
                    # sumexp row is copied to partition 0 (recip_approx_fast
                    # misreads non-zero base partitions on HW).
                    stg = stgpool.tile([HD, RQ], F32, tag="stg", name="stg")
                    nc.vector.tensor_copy(stg[:], avs[j][0:HD, :])
                    sume = stgpool.tile([1, RQ], F32, tag="sume", name="sume")
                    nc.vector.tensor_copy(sume[:], avs[j][HD:HD + 1, :])
                    rec = rcpool.tile([1, RQ], F32, tag="rec", name="rec")
                    nc.vector.reciprocal_approx_fast(rec[:], sume[:])
                    rbc = rcpool.tile([64, RQ], F32, tag="rbc", name="rbc")
                    nc.gpsimd.partition_broadcast(rbc[:], rec[:])
                    nc.vector.tensor_tensor(
                        aoT[64 * j:64 * j + 64, ft * RQ:(ft + 1) * RQ],
                        stg[:], rbc[:], MUL)

        # ---------------- Phase C: output projection -----------------------
        with tc.tile_pool(name="yb", bufs=2) as ypool, \
             tc.tile_pool(name="ypp", bufs=2, space="PSUM") as ypp:
            for qt in range(NQT if "C" in phases else 0):
                yp = ypp.tile([P, D], F32, tag="yp", name="yp")
                for nb in range(2):
                    out = yp[:, nb * 512:(nb + 1) * 512]
                    for f in range(NF):
                        nc.tensor.matmul(
                            out,
                            lhsT=aoT[:, f * RQ + qt * P: f * RQ + (qt + 1) * P],
                            rhs=w0t[f][:, nb * 512:(nb + 1) * 512],
                            start=(f == 0), stop=False)
                    nc.tensor.matmul(
                        out, lhsT=ones[:],
                        rhs=brow["b0"][:, nb * 512:(nb + 1) * 512],
                        start=False, stop=True)
                ysb = ypool.tile([P, D], F32, tag="y", name="ysb")
                nc.vector.tensor_copy(ysb[:], yp[:])
                nc.sync.dma_start(y_d[qt * P:(qt + 1) * P, :], ysb[:])

    nc.compile()
    _CACHE[key] = nc
    return nc


def _prep_shared(Wq, bq, Wk, bk, Wv, bv, W0, b0):
    perm, cosT, sinT, ident = _host_prep()
    as_bf = lambda a: np.ascontiguousarray(a).astype(NPBF16)
    _CACHE["shared"] = {
        "wq": as_bf(Wq[:, perm]), "wk": as_bf(Wk[:, perm]),
        "wv": as_bf(Wv), "w0": as_bf(W0),
        "bq": as_bf(bq[perm][None, :]), "bk": as_bf(bk[perm][None, :]),
        "bv": as_bf(bv[None, :]), "b0": as_bf(b0[None, :]),
        "ident": ident,
    }


def _make_in_maps(x):
    perm, cosT, sinT, ident = _host_prep()
    shared = _CACHE["shared"]
    in_maps = []
    for c in range(8):
        b, half = c // 2, c % 2
        order = np.r_[half * RQ:(half + 1) * RQ,
                      (1 - half) * RQ:(2 - half) * RQ]
        xb = x[b][order]                                   # [2048, 1024]
        m = dict(shared)
        m["xT"] = np.ascontiguousarray(xb.T).astype(NPBF16)
        m["cosT"] = np.ascontiguousarray(cosT[order])
        m["sinT"] = np.ascontiguousarray(sinT[order])
        in_maps.append(m)
    return in_maps


def kernel_results(x, Wq, bq, Wk, bk, Wv, bv, W0, b0, trace=False,
                   **trace_kwargs):
    """Run on 8 cores; returns (full output [B,S,D] f32, BassKernelResults)."""
    from concourse.bass_utils import run_bass_kernel_spmd
    x = np.asarray(x, np.float32)
    _prep_shared(np.asarray(Wq, np.float32), np.asarray(bq, np.float32),
                 np.asarray(Wk, np.float32), np.asarray(bk, np.float32),
                 np.asarray(Wv, np.float32), np.asarray(bv, np.float32),
                 np.asarray(W0, np.float32), np.asarray(b0, np.float32))
    nc = _build_module()
    in_maps = _make_in_maps(x)
    res = run_bass_kernel_spmd(nc, in_maps, list(range(8)), trace=trace,
                               **trace_kwargs)
    out = np.empty((B, S, D), np.float32)
    for c in range(8):
        b, half = c // 2, c % 2
        out[b, half * RQ:(half + 1) * RQ] = res.results[c]["y"]
    return out, res


def kernel(x, Wq, bq, Wk, bk, Wv, bv, W0, b0):
    out, _ = kernel_results(x, Wq, bq, Wk, bk, Wv, bv, W0, b0)
    return out


# revision 22
# speedup vs baseline: 1.1970x; 1.1970x over previous
"""Multi-head attention (B=4, S=2048, D=1024, H=16) on 8 TRN2 NeuronCores.

Sharding: row-parallel over (batch, seq-half): core c owns batch b=c//2 and
query rows [half*1024, half*1024+1024), half=c%2. K/V are computed for the
full batch on each core (duplicated compute, zero collectives). Each core's
output rows are final => no all-reduce; host just stacks the 8 outputs.

Device pipeline per core (uniform SPMD; per-core differences live in in_maps):
  - QKV projections row-major (lhsT = xT tiles, rhs = W tiles); bias folded in
    as a K=1 ones-row matmul in the same PSUM accumulation group.
  - RoPE in row-major layout. Wq/Wk columns are pre-permuted per head to
    [evens | odds] so interleaved RoPE becomes rotate-half on free-dim slices.
  - PE transposes (regular matmul against identity) -> feature-major qT/kT.
  - scoresT[kpos, q] = (kT_h slice as lhsT) x (qT_h as rhs), K=64; exp on
    ScalarE straight from PSUM with fused 1/sqrt(64) scale; bf16 out.
  - AV: outT[d, q] accumulated over kpos tiles with v row-major as stationary;
    a ones column appended to v yields the softmax denominator for free.
  - normalize via VectorE reciprocal + GPSIMD partition-broadcast.
  - y = (aoT slices as lhsT) x W0 + b0 (same ones trick), f32 out, DMA.
"""

import numpy as np
import ml_dtypes

B, S, D, H = 4, 2048, 1024, 16
HD = D // H          # 64
P = 128
RQ = 1024            # query rows per core
RK = 2048            # key rows per core (full batch)
NQT = RQ // P        # 8
NKT = RK // P        # 16
NF = D // P          # 8
NPBF16 = np.dtype(ml_dtypes.bfloat16)

_CACHE = {}


def _host_prep():
    if "prep" in _CACHE:
        return _CACHE["prep"]
    perm = np.empty(D, np.int64)
    for h in range(H):
        base = h * HD
        perm[base:base + 32] = base + np.arange(0, HD, 2)
        perm[base + 32:base + 64] = base + np.arange(1, HD, 2)
    inv_freq = 1.0 / (10000.0 ** (np.arange(0, D, 2, dtype=np.float32) / D))
    pos = np.arange(S, dtype=np.float32)
    ang = pos[:, None] * inv_freq[None, :]          # [S, 512]
    cosT = np.cos(ang).astype(NPBF16)
    sinT = np.sin(ang).astype(NPBF16)
    ident = np.eye(P, dtype=NPBF16)
    _CACHE["prep"] = (perm, cosT, sinT, ident)
    return _CACHE["prep"]


def _build_module(phases="ABC"):
    key = f"nc_{phases}"
    if key in _CACHE:
        return _CACHE[key]
    import concourse.bacc as bacc
    import concourse.mybir as mybir
    import concourse.tile as tile
    from contextlib import ExitStack

    BF = mybir.dt.bfloat16
    F32 = mybir.dt.float32
    Exp = mybir.ActivationFunctionType.Exp
    MUL = mybir.AluOpType.mult
    SUB = mybir.AluOpType.subtract
    ADD = mybir.AluOpType.add

    nc = bacc.Bacc("TRN2", target_bir_lowering=False, debug=False,
                   enable_asserts=False, num_devices=8)

    xT_d = nc.dram_tensor("xT", [D, RK], BF, kind="ExternalInput").ap()
    cos_d = nc.dram_tensor("cosT", [RK, D // 2], BF, kind="ExternalInput").ap()
    sin_d = nc.dram_tensor("sinT", [RK, D // 2], BF, kind="ExternalInput").ap()
    wq_d = nc.dram_tensor("wq", [D, D], BF, kind="ExternalInput").ap()
    wk_d = nc.dram_tensor("wk", [D, D], BF, kind="ExternalInput").ap()
    wv_d = nc.dram_tensor("wv", [D, D], BF, kind="ExternalInput").ap()
    w0_d = nc.dram_tensor("w0", [D, D], BF, kind="ExternalInput").ap()
    bq_d = nc.dram_tensor("bq", [1, D], BF, kind="ExternalInput").ap()
    bk_d = nc.dram_tensor("bk", [1, D], BF, kind="ExternalInput").ap()
    bv_d = nc.dram_tensor("bv", [1, D], BF, kind="ExternalInput").ap()
    b0_d = nc.dram_tensor("b0", [1, D], BF, kind="ExternalInput").ap()
    id_d = nc.dram_tensor("ident", [P, P], BF, kind="ExternalInput").ap()
    y_d = nc.dram_tensor("y", [RQ, D], F32, kind="ExternalOutput").ap()

    def by_head(ap, inner):
        # [P, H*inner] -> [P, H, inner]
        return ap.rearrange("p (h e) -> p h e", h=H)

    with tile.TileContext(nc) as tc, ExitStack() as ctx:
        cpool = ctx.enter_context(tc.tile_pool(name="const", bufs=1))
        ident = cpool.tile([P, P], BF)
        nc.sync.dma_start(ident[:], id_d)
        ones = cpool.tile([1, P], BF)
        nc.vector.memset(ones[:], 1.0)
        brow = {}
        for nm, d in (("bq", bq_d), ("bk", bk_d), ("bv", bv_d), ("b0", b0_d)):
            t = cpool.tile([1, D], BF, tag=f"brow_{nm}", name=f"brow_{nm}")
            nc.sync.dma_start(t[:], d)
            brow[nm] = t
        kT = cpool.tile([P, NF * RK], BF)       # [128, 8*2048]
        # qz: per-head zero-padded qT. Head h data lives at partitions
        # [64*(h%2), 64*(h%2)+64), cols [h*RQ, (h+1)*RQ); the other 64
        # partitions stay zero so scores matmuls contract over K=128
        # (full PE array => no HAM utilization throttle).
        qz = cpool.tile([P, H * RQ], BF)
        for j in range(2):
            zrows = qz[64 * (1 - j):64 * (1 - j) + 64, :].rearrange(
                "p (h r) -> p h r", h=H)[:, j::2, :]
            nc.gpsimd.memset(zrows, 0.0)

        vpool = ctx.enter_context(tc.tile_pool(name="v", bufs=NKT))
        v_tiles = []

        # ---------------- Phase A: projections + rope + transpose ----------
        with tc.tile_pool(name="x", bufs=NF) as xpool, \
             tc.tile_pool(name="w", bufs=2 * NF) as wpool, \
             tc.tile_pool(name="cs", bufs=2) as cspool, \
             tc.tile_pool(name="ro", bufs=2) as ropool, \
             tc.tile_pool(name="pjp", bufs=2, space="PSUM") as pjp, \
             tc.tile_pool(name="tpp", bufs=2, space="PSUM") as tpp:

            xt = []
            for i in range(NF):
                t = xpool.tile([P, RK], BF, tag="xT", name=f"xT{i}")
                nc.sync.dma_start(t[:], xT_d[i * P:(i + 1) * P, :])
                xt.append(t)
            def _load_w(nm, d):
                tiles = []
                for i in range(NF):
                    t = wpool.tile([P, D], BF, tag="w", name=f"{nm}_{i}")
                    nc.sync.dma_start(t[:], d[i * P:(i + 1) * P, :])
                    tiles.append(t)
                return tiles

            def _proj(rt, wtiles, btile):
                """Row-tile rt -> PSUM [128 rows, 1024 feats], bias included."""
                pt = pjp.tile([P, D], F32, tag="pj", name="pj")
                for nb in range(2):
                    out = pt[:, nb * 512:(nb + 1) * 512]
                    for kin in range(NF):
                        nc.tensor.matmul(
                            out,
                            lhsT=xt[kin][:, rt * P:(rt + 1) * P],
                            rhs=wtiles[kin][:, nb * 512:(nb + 1) * 512],
                            start=(kin == 0), stop=False)
                    nc.tensor.matmul(
                        out, lhsT=ones[:],
                        rhs=btile[:, nb * 512:(nb + 1) * 512],
                        start=False, stop=True)
                return pt

            def _rope_transpose(rt, pt, dst):
                """RoPE psum row-tile -> bf16; PE-transpose into dst cols rt."""
                ct = cspool.tile([P, D // 2], BF, tag="cos", name="cos")
                st = cspool.tile([P, D // 2], BF, tag="sin", name="sin")
                nc.sync.dma_start(ct[:], cos_d[rt * P:(rt + 1) * P, :])
                nc.sync.dma_start(st[:], sin_d[rt * P:(rt + 1) * P, :])
                rp = ropool.tile([P, D], BF, tag="roped", name="roped")
                t1 = ropool.tile([P, D // 2], F32, tag="t1", name="t1")
                t2 = ropool.tile([P, D // 2], F32, tag="t2", name="t2")

                def EO(ap, j):  # j=0 evens-half, j=1 odds-half per head
                    return ap.rearrange("p (h t e) -> p h t e", h=H, t=2)[:, :, j:j + 1, :]

                ch = by_head(ct[:], 32)
                sh = by_head(st[:], 32)
                t1h = by_head(t1[:], 32)
                t2h = by_head(t2[:], 32)
                nc.vector.tensor_tensor(t1h, EO(pt[:], 0), ch, MUL)
                nc.vector.tensor_tensor(t2h, EO(pt[:], 1), sh, MUL)
                nc.vector.tensor_tensor(EO(rp[:], 0), t1h, t2h, SUB)
                nc.vector.tensor_tensor(t1h, EO(pt[:], 1), ch, MUL)
                nc.vector.tensor_tensor(t2h, EO(pt[:], 0), sh, MUL)
                nc.vector.tensor_tensor(EO(rp[:], 1), t1h, t2h, ADD)
                if dst is None:
                    # q: PE-transpose 8 slices, scatter head halves into qz
                    for g in range(2):
                        tp = tpp.tile([P, 4 * P], F32, tag="tp", name="tp")
                        for j in range(4):
                            f = g * 4 + j
                            nc.tensor.matmul(
                                tp[:, j * P:(j + 1) * P],
                                lhsT=rp[:, f * P:(f + 1) * P], rhs=ident[:],
                                start=True, stop=True)
                        # head 8g+2i+j sits at cols (8g+2i+j)*RQ, partitions 64j
                        for j in range(2):
                            out = qz[64 * j:64 * j + 64, :].rearrange(
                                "p (h r) -> p h r", h=H)[
                                :, 8 * g + j:min(8 * g + j + 8, H):2,
                                rt * P:(rt + 1) * P]
                            nc.vector.tensor_copy(
                                out, tp[64 * j:64 * j + 64, :].rearrange(
                                    "p (i r) -> p i r", i=4))
                else:
                    # k: PE-transpose 8 slices, 4 per psum tile, one evict
                    for g in range(2):
                        tp = tpp.tile([P, 4 * P], F32, tag="tp", name="tp")
                        for j in range(4):
                            f = g * 4 + j
                            nc.tensor.matmul(
                                tp[:, j * P:(j + 1) * P],
                                lhsT=rp[:, f * P:(f + 1) * P], rhs=ident[:],
                                start=True, stop=True)
                        out = dst[:].rearrange("p (f r) -> p f r", f=NF)[
                            :, g * 4:(g + 1) * 4, rt * P:(rt + 1) * P]
                        nc.vector.tensor_copy(
                            out, tp[:].rearrange("p (j r) -> p j r", j=4))

            def _vtile(rt, wv_t):
                pt = _proj(rt, wv_t, brow["bv"])
                vt = vpool.tile([P, H * (HD + 1)], BF, tag="v", name=f"v{rt}")
                vh = vt[:].rearrange("p (h e) -> p h e", h=H)
                nc.vector.tensor_copy(vh[:, :, 0:HD], by_head(pt[:], HD))
                nc.vector.memset(vh[:, :, HD:HD + 1], 1.0)
                v_tiles.append(vt)

            wq_t = _load_w("wq", wq_d)
            for rt in range(NQT):
                _rope_transpose(rt, _proj(rt, wq_t, brow["bq"]), None)
            wk_t = _load_w("wk", wk_d)
            wv_t = _load_w("wv", wv_d)
            for rt in range(NKT):
                _rope_transpose(rt, _proj(rt, wk_t, brow["bk"]), kT)
                _vtile(rt, wv_t)

        # ---------------- Phase B: attention, head pairs -------------------
        w0pool = ctx.enter_context(tc.tile_pool(name="w0", bufs=NF))
        aoT = w0pool.tile([P, NF * RQ], BF, tag="aoT", name="aoT", bufs=1)
        w0t = []
        for i in range(NF):
            t = w0pool.tile([P, D], BF, tag="w0", name=f"w0_{i}")
            nc.sync.dma_start(t[:], w0_d[i * P:(i + 1) * P, :])
            w0t.append(t)

        with tc.tile_pool(name="at", bufs=8) as apool, \
             tc.tile_pool(name="rec", bufs=2) as rcpool, \
             tc.tile_pool(name="stg", bufs=4) as stgpool, \
             tc.tile_pool(name="scp", bufs=2, space="PSUM") as scp, \
             tc.tile_pool(name="avp", bufs=2, space="PSUM") as avp:

            def _scores(ft, kt):
                """Scores + exp for both heads of pair ft at key tile kt.
                K=128 full-array matmuls: the complementary head's rows in
                qz are zero, so they contribute nothing."""
                ats = []
                for j in range(2):
                    h = 2 * ft + j
                    sp = scp.tile([P, RQ], F32, tag="sc", name="sc")
                    for nb in range(2):
                        nc.tensor.matmul(
                            sp[:, nb * 512:(nb + 1) * 512],
                            lhsT=kT[:, ft * RK + kt * P: ft * RK + (kt + 1) * P],
                            rhs=qz[:, h * RQ + nb * 512: h * RQ + (nb + 1) * 512],
                            start=True, stop=True)
                    at = apool.tile([P, RQ], BF, tag="at", name="at")
                    nc.scalar.activation(at[:], sp[:], Exp, scale=0.125)
                    ats.append(at)
                return ats

            def _av(hp, kt, avs, ats):
                for j in range(2):
                    h = 2 * hp + j
                    for nb in range(2):
                        nc.tensor.matmul(
                            avs[j][0:HD + 1, nb * 512:(nb + 1) * 512],
                            lhsT=v_tiles[kt][:, h * (HD + 1):(h + 1) * (HD + 1)],
                            rhs=ats[j][:, nb * 512:(nb + 1) * 512],
                            start=(kt == 0), stop=(kt == NKT - 1))

            for hp in range(H // 2 if "B" in phases else 0):
                ft = hp            # feature tile holding heads 2hp, 2hp+1
                avs = [avp.tile([P, RQ], F32, tag="av", name=f"av{_j}") for _j in range(2)]
                # software pipeline: scores/exp run one kt ahead of AV
                prev = _scores(ft, 0)
                for kt in range(1, NKT):
                    cur = _scores(ft, kt)
                    _av(hp, kt - 1, avs, prev)
                    prev = cur
                _av(hp, NKT - 1, avs, prev)
                for j in range(2):
                    # cheap copies release the AV psum tile; the rest of the
                    # normalize chain runs off the PE critical path. The
                    # sumexp row is copied to partition 0 (recip_approx_fast
                    # misreads non-zero base partitions on HW).
                    stg = stgpool.tile([HD, RQ], F32, tag="stg", name="stg")
                    nc.vector.tensor_copy(stg[:], avs[j][0:HD, :])
                    sume = stgpool.tile([1, RQ], F32, tag="sume", name="sume")
                    nc.vector.tensor_copy(sume[:], avs[j][HD:HD + 1, :])
                    rec = rcpool.tile([1, RQ], F32, tag="rec", name="rec")
                    nc.vector.reciprocal_approx_fast(rec[:], sume[:])
                    rbc = rcpool.tile([64, RQ], F32, tag="rbc", name="rbc")
                    nc.gpsimd.partition_broadcast(rbc[:], rec[:])
                    nc.vector.tensor_tensor(
                        aoT[64 * j:64 * j + 64, ft * RQ:(ft + 1) * RQ],
                        stg[:], rbc[:], MUL)

        # ---------------- Phase C: output projection -----------------------
        with tc.tile_pool(name="yb", bufs=2) as ypool, \
             tc.tile_pool(name="ypp", bufs=2, space="PSUM") as ypp:
            for qt in range(NQT if "C" in phases else 0):
                yp = ypp.tile([P, D], F32, tag="yp", name="yp")
                for nb in range(2):
                    out = yp[:, nb * 512:(nb + 1) * 512]
                    for f in range(NF):
                        nc.tensor.matmul(
                            out,
                            lhsT=aoT[:, f * RQ + qt * P: f * RQ + (qt + 1) * P],
                            rhs=w0t[f][:, nb * 512:(nb + 1) * 512],
                            start=(f == 0), stop=False)
                    nc.tensor.matmul(
                        out, lhsT=ones[:],
                        rhs=brow["b0"][:, nb * 512:(nb + 1) * 512],
                        start=False, stop=True)
                ysb = ypool.tile([P, D], F32, tag="y", name="ysb")
                nc.vector.tensor_copy(ysb[:], yp[:])
                nc.sync.dma_start(y_d[qt * P:(qt + 1) * P, :], ysb[:])

    nc.compile()
    _CACHE[key] = nc
    return nc


def _prep_shared(Wq, bq, Wk, bk, Wv, bv, W0, b0):
    perm, cosT, sinT, ident = _host_prep()
    as_bf = lambda a: np.ascontiguousarray(a).astype(NPBF16)
    _CACHE["shared"] = {
        "wq": as_bf(Wq[:, perm]), "wk": as_bf(Wk[:, perm]),
        "wv": as_bf(Wv), "w0": as_bf(W0),
        "bq": as_bf(bq[perm][None, :]), "bk": as_bf(bk[perm][None, :]),
        "bv": as_bf(bv[None, :]), "b0": as_bf(b0[None, :]),
        "ident": ident,
    }


def _make_in_maps(x):
    perm, cosT, sinT, ident = _host_prep()
    shared = _CACHE["shared"]
    in_maps = []
    for c in range(8):
        b, half = c // 2, c % 2
        order = np.r_[half * RQ:(half + 1) * RQ,
                      (1 - half) * RQ:(2 - half) * RQ]
        xb = x[b][order]                                   # [2048, 1024]
        m = dict(shared)
        m["xT"] = np.ascontiguousarray(xb.T).astype(NPBF16)
        m["cosT"] = np.ascontiguousarray(cosT[order])
        m["sinT"] = np.ascontiguousarray(sinT[order])
        in_maps.append(m)
    return in_maps


def kernel_results(x, Wq, bq, Wk, bk, Wv, bv, W0, b0, trace=False,
                   **trace_kwargs):
    """Run on 8 cores; returns (full output [B,S,D] f32, BassKernelResults)."""
    from concourse.bass_utils import run_bass_kernel_spmd
    x = np.asarray(x, np.float32)
    _prep_shared(np.asarray(Wq, np.float32), np.asarray(bq, np.float32),
                 np.asarray(Wk, np.float32), np.asarray(bk, np.float32),
                 np.asarray(Wv, np.float32), np.asarray(bv, np.float32),
                 np.asarray(W0, np.float32), np.asarray(b0, np.float32))
    nc = _build_module()
    in_maps = _make_in_maps(x)
    res = run_bass_kernel_spmd(nc, in_maps, list(range(8)), trace=trace,
                               **trace_kwargs)
    out = np.empty((B, S, D), np.float32)
    for c in range(8):
        b, half = c // 2, c % 2
        out[b, half * RQ:(half + 1) * RQ] = res.results[c]["y"]
    return out, res


def kernel(x, Wq, bq, Wk, bk, Wv, bv, W0, b0):
    out, _ = kernel_results(x, Wq, bq, Wk, bk, Wv, bv, W0, b0)
    return out
